# revision 38
# baseline (speedup 1.0000x reference)
"""Builder + host glue for the ViT attention kernel on 8 trn2 cores.

Reference computation (per batch b):
    qkv = x @ w_qkv.T ; q,k,v split; per head: softmax(q k^T / sqrt(dh)) v
    out = attn @ w_out.T + b_out

Sharding: data-parallel over batch (8 batches per core).

Schedule: group-major. QKV projection runs one 256-col weight group at a
time across all four token chunks, so the head of the kernel only needs
x plus a single weight group in SBUF before the PE saturates (the DMA
feed is pure-bandwidth-bound at ~330 GB/s; front-loading all twelve
groups would stall the PE for ~7us). Attention pairs for group g are
threaded through phase g+1, v projections ride along as soon as wv
lands, and the out-projections fill the tail while the last pairs'
softmax chains drain on Scalar/Vector/GpSimd.

Inputs are pre-arranged on the host into the exact per-partition SBUF
layout of each destination tile, so every DMA is 128 fully contiguous
descriptors.
"""

import numpy as np
import ml_dtypes

import concourse.bass as bass
import concourse.tile as tile
from concourse import bacc, mybir
from concourse.bass_utils import run_bass_kernel_spmd

P = 128
B, N, D = 64, 197, 768
H, DH = 12, 64
NCORES = 8
BPC = B // NCORES          # 8 batches per core
T = BPC * N                # 1576 tokens per core
KT = D // P                # 6 contraction tiles
NPAIR = H // 2             # 6 head pairs
SCALE = DH ** -0.5
VW = (DH + 1) * H          # 780: v columns, ones col LAST per head
N2 = 2 * N                 # 394
JT1 = N - P                # 69: second j-tile size

BF = mybir.dt.bfloat16
F32 = mybir.dt.float32
EXP = mybir.ActivationFunctionType.Exp
IDENT = mybir.ActivationFunctionType.Identity


def build_nc():
    nc = bacc.Bacc(
        "TRN2", target_bir_lowering=False, debug=False, num_devices=NCORES
    )
    d_xc0 = [
        nc.dram_tensor(f"xc0_{i}", [P, 2 * 394], BF, kind="ExternalInput").ap()
        for i in range(3)
    ]
    d_xc = [
        nc.dram_tensor(f"xc{c}", [P, KT * 394], BF, kind="ExternalInput").ap()
        for c in (1, 2, 3)
    ]
    d_wqk0 = [
        nc.dram_tensor(f"wqk0_{i}", [P, 3 * 256], BF, kind="ExternalInput").ap()
        for i in range(2)
    ]
    d_wqk = [
        nc.dram_tensor(f"wqk{g}", [P, KT * 256], BF, kind="ExternalInput").ap()
        for g in range(1, 6)
    ]
    d_wva = nc.dram_tensor("wva", [P, KT * 512], BF, kind="ExternalInput").ap()
    d_wvb = nc.dram_tensor("wvb", [P, KT * 256], BF, kind="ExternalInput").ap()
    d_wo = [
        nc.dram_tensor(f"wo{i}", [P, 3 * D], BF, kind="ExternalInput").ap()
        for i in range(2)
    ]
    bias = nc.dram_tensor("bias", [P, KT], F32, kind="ExternalInput").ap()
    outT = nc.dram_tensor("outT", [D, T], BF, kind="ExternalOutput").ap()

    with tile.TileContext(nc) as tc:
        with (
            tc.tile_pool(name="big", bufs=1) as big,
            tc.tile_pool(name="exp", bufs=8) as sb_exp,
            tc.tile_pool(name="rec", bufs=6) as sb_rec,
            tc.tile_pool(name="bsb", bufs=6) as sb_bsb,
            tc.tile_pool(name="osb", bufs=6) as sb_osb,
            tc.tile_pool(name="ps_pj", bufs=2, space="PSUM") as ps_pj,
            tc.tile_pool(name="ps_att", bufs=6, space="PSUM") as ps_att,
        ):
            # ---- persistent SBUF tiles ----------------------------------
            bias_sb = big.tile([P, KT], F32, tag="bias")
            # x: chunk tiles; c0 split in 3 (2 k-tiles each) for fast start
            x_c0 = [big.tile([P, 2, 394], BF, tag=f"xc0_{i}", name=f"xc0_{i}") for i in range(3)]
            x_c = [big.tile([P, KT, 394], BF, tag=f"xc{c}", name=f"xc{c}") for c in (1, 2, 3)]
            # wqk: per 256-col group; g0 split in 2 (3 k-tiles each)
            wqk_g0 = [big.tile([P, 3, 256], BF, tag=f"wqk0_{i}", name=f"wqk0_{i}") for i in range(2)]
            wqk_g = {
                g: big.tile([P, KT, 256], BF, tag=f"wqk{g}", name=f"wqk{g}")
                for g in range(1, 6)
            }
            # wv split by output col-group: piece 0 = cols 0:512, piece 1 = 512:768
            wv_sb = [
                big.tile([P, KT, 512], BF, tag="wva", name="wva"),
                big.tile([P, KT, 256], BF, tag="wvb", name="wvb"),
            ]
            wo_sb = [big.tile([P, 3, D], BF, tag=f"wo{i}", name=f"wo{i}") for i in range(2)]
            # warmup dummies
            wdum = big.tile([P, P], BF, tag="wdum")
            xdum = big.tile([P, 512], BF, tag="xdum")

            qk_sb = [big.tile([P, T], BF, tag=f"qk{m}", name=f"qk{m}") for m in range(2 * NPAIR)]
            v_sb = [big.tile([P, VW], BF, tag=f"v{i}", name=f"v{i}") for i in range(2 * BPC)]
            at_sb = [
                [big.tile([P, N2], BF, tag=f"at{p}_{b2}", name=f"at{p}_{b2}") for b2 in range(BPC // 2)]
                for p in range(NPAIR)
            ]

            # ---- input DMAs (merged, orchestrated) ----------------------
            def pk(src, kp, cols):
                # dram [p, k*cols] (partition-contiguous) -> [p, k, c]
                return src.rearrange("p (k c) -> p k c", k=kp, c=cols)

            # DMA ring roles (HWDGE DMAs BLOCK their issuing engine until
            # completion): scalar carries the three small x_c0 pieces in
            # parallel with sync's FIFO, which holds everything else in
            # exact consumption order, then the out tiles.
            for i in range(3):
                nc.scalar.dma_start(x_c0[i][:], pk(d_xc0[i], 2, 394))
            for i in range(2):
                nc.sync.dma_start(wqk_g0[i][:], pk(d_wqk0[i], 3, 256))
            nc.sync.dma_start(x_c[0][:], pk(d_xc[0], KT, 394))
            nc.sync.dma_start(wqk_g[1][:], pk(d_wqk[0], KT, 256))
            nc.sync.dma_start(x_c[1][:], pk(d_xc[1], KT, 394))
            nc.sync.dma_start(x_c[2][:], pk(d_xc[2], KT, 394))
            nc.sync.dma_start(wv_sb[0][:], pk(d_wva, KT, 512))
            nc.sync.dma_start(wv_sb[1][:], pk(d_wvb, KT, 256))
            for g in range(2, 6):
                nc.sync.dma_start(wqk_g[g][:], pk(d_wqk[g - 1], KT, 256))
            for i in range(2):
                nc.sync.dma_start(wo_sb[i][:], pk(d_wo[i], 3, D))
            nc.sync.dma_start(bias_sb[:], bias)

            # ---- init memsets -------------------------------------------
            nc.vector.memset(wdum[:], 0.0)
            nc.vector.memset(xdum[:], 0.0)
            for i in range(2 * BPC):
                ones_cols = v_sb[i][:].rearrange("p (h c) -> p h c", c=DH + 1)[
                    :, :, DH : DH + 1
                ]
                nc.gpsimd.memset(ones_cols, 1.0)

            # ---- AP helpers ---------------------------------------------
            def x_ap(k, t0, tl):
                c = t0 // 394
                o = t0 - c * 394
                if c == 0:
                    return x_c0[k // 2][:, k % 2, o : o + tl]
                return x_c[c - 1][:, k, o : o + tl]

            def wqk_ap(k, m):
                # wqkT columns are interleaved per pair: group p holds
                # [q_p (128 cols), k_p (128 cols)] so each pair-slot of the
                # schedule consumes exactly one 256-col weight group.
                g = m % NPAIR
                o = P if m >= NPAIR else 0
                if g == 0:
                    return wqk_g0[k // 3][:, k % 3, o : o + P]
                return wqk_g[g][:, k, o : o + P]

            def wv_ap(k, c0, cl):
                if c0 == 0:
                    return wv_sb[0][:, k, 0:cl]
                return wv_sb[1][:, k, 0:cl]

            def wo_ap(k, m):
                return wo_sb[k // 3][:, k % 3, m * P : (m + 1) * P]

            # ---- engine alternation counters ----------------------------
            alt = {"cast": 0, "act": 0}

            # ---- unit emitters ------------------------------------------
            warm_n = [0]

            def warmup(n):
                psum = ps_pj.tile([P, 512], F32, tag="pj", name="warm")[:, :394]
                for r in range(n):
                    nc.tensor.matmul(
                        psum, wdum[:], xdum[:, 0:394], start=True, stop=True
                    )
                warm_n[0] += 1
                wout = big.tile([P, 8], F32, tag=f"warmout{warm_n[0]}", name="wout")
                nc.vector.tensor_copy(out=wout[:], in_=psum[:, 0:8])

            def qkv_proj(m, c, cast_eng=None):
                t0 = c * 394
                psum = ps_pj.tile([P, 512], F32, tag="pj", name="pj")[:, :394]
                for k in range(KT):
                    nc.tensor.matmul(
                        psum,
                        wqk_ap(k, m),
                        x_ap(k, t0, 394),
                        start=(k == 0),
                        stop=(k == KT - 1),
                    )
                out = qk_sb[m][:, t0 : t0 + 394]
                if cast_eng is None:
                    alt["cast"] += 1
                    cast_eng = "v" if alt["cast"] % 2 == 0 else "s"
                if cast_eng == "v":
                    nc.vector.tensor_copy(out=out, in_=psum)
                else:
                    nc.scalar.copy(out=out, in_=psum)

            def vproj(b, jt, cg, copy_eng=None):
                c0, cl = (0, 512) if cg == 0 else (512, 256)
                r0 = b * N + jt * P
                rl = P if jt == 0 else JT1
                i = 2 * b + jt
                psum = ps_pj.tile([P, 512], F32, tag="pj", name="pjv")[:rl, :cl]
                for k in range(KT):
                    nc.tensor.matmul(
                        psum,
                        x_ap(k, r0, rl),
                        wv_ap(k, c0, cl),
                        start=(k == 0),
                        stop=(k == KT - 1),
                    )
                hs = c0 // DH
                nh = cl // DH
                out_ap = v_sb[i][:rl].rearrange("p (h c) -> p h c", c=DH + 1)[
                    :, hs : hs + nh, 0:DH
                ]
                if copy_eng is None:
                    alt["cast"] += 1
                    copy_eng = "v" if alt["cast"] % 2 == 0 else "s"
                if copy_eng == "s":
                    nc.scalar.copy(
                        out=out_ap, in_=psum.rearrange("p (h c) -> p h c", c=DH)
                    )
                else:
                    nc.vector.tensor_copy(
                        out=out_ap, in_=psum.rearrange("p (h c) -> p h c", c=DH)
                    )

            def outproj(b2, m, t0, tl):
                psum = ps_pj.tile([P, 512], F32, tag="pj", name="op")[:, :tl]
                for p in range(NPAIR):
                    nc.tensor.matmul(
                        psum,
                        wo_ap(p, m),
                        at_sb[p][b2][:, t0 : t0 + tl],
                        start=(p == 0),
                        stop=(p == NPAIR - 1),
                    )
                osb = sb_osb.tile([P, 394], BF, tag="osb", name="osb")[:, :tl]
                alt["act"] += 1
                if alt["act"] % 2 == 0:
                    nc.scalar.activation(osb, psum, IDENT, bias=bias_sb[:, m : m + 1])
                else:
                    nc.vector.tensor_scalar_add(osb, psum, bias_sb[:, m : m + 1])
                nc.sync.dma_start(
                    outT[m * P : (m + 1) * P, b2 * N2 + t0 : b2 * N2 + t0 + tl], osb
                )

            pair_state = {}

            def pair_qk(b, p):
                tb = b * N
                qT = qk_sb[p]
                kTt = qk_sb[NPAIR + p]
                expT = []
                for h in (0, 1):
                    e0 = DH * h
                    ps_s = ps_att.tile([P, N2], F32, tag="att", name="sc")
                    nc.tensor.matmul(
                        ps_s[0:P, 0:N],
                        kTt[e0 : e0 + DH, tb : tb + P],
                        qT[e0 : e0 + DH, tb : tb + N],
                        start=True,
                        stop=True,
                        tile_position=(e0, 0),
                    )
                    nc.tensor.matmul(
                        ps_s[0:JT1, N:N2],
                        kTt[e0 : e0 + DH, tb + P : tb + N],
                        qT[e0 : e0 + DH, tb : tb + N],
                        start=True,
                        stop=True,
                        tile_position=(e0, 0),
                    )
                    e = sb_exp.tile([P, N2], BF, tag="expT", name="expT")
                    nc.scalar.activation(e[:], ps_s[:], EXP)
                    expT.append(e)
                pair_state[(b, p)] = expT

            def pair_av(b, p):
                expT = pair_state.pop((b, p))
                pso = ps_att.tile([P, N2], F32, tag="att", name="o")[0 : DH + 1, :]
                for h in (0, 1):
                    vc = (DH + 1) * (2 * p + h)
                    nc.tensor.matmul(
                        pso[:, N * h : N * h + N],
                        v_sb[2 * b][0:P, vc : vc + DH + 1],
                        expT[h][0:P, 0:N],
                        start=True,
                        stop=False,
                    )
                    nc.tensor.matmul(
                        pso[:, N * h : N * h + N],
                        v_sb[2 * b + 1][0:JT1, vc : vc + DH + 1],
                        expT[h][0:JT1, N:N2],
                        start=False,
                        stop=True,
                    )
                # ones col is LAST per head -> s row = psum partition 64.
                # custom DVE needs base-0 SBUF input: stage s via a copy
                # (alternating engines), then approx-reciprocal.
                s_sb = sb_rec.tile([1, N2], F32, tag="s_sb", name="s_sb")
                alt["cast"] += 1
                if alt["cast"] % 2 == 0:
                    nc.vector.tensor_copy(out=s_sb[:], in_=pso[DH : DH + 1, :])
                else:
                    nc.scalar.copy(out=s_sb[:], in_=pso[DH : DH + 1, :])
                rec = sb_rec.tile([1, N2], F32, tag="rec", name="rec")
                nc.vector.reciprocal_approx_fast(out=rec[:], in_=s_sb[:])
                bsb = sb_bsb.tile([DH, N2], F32, tag="bsb", name="bsb")
                nc.gpsimd.partition_broadcast(bsb[:], rec[:])
                for h in (0, 1):
                    nc.vector.tensor_mul(
                        out=at_sb[p][b // 2][
                            DH * h : DH * h + DH, N * (b % 2) : N * (b % 2) + N
                        ],
                        in0=pso[0:DH, N * h : N * h + N],
                        in1=bsb[:, N * h : N * h + N],
                    )

            # ---- static schedule ----------------------------------------
            # Group-major: phase g emits the 8 qkv units of group g (q and
            # k slots, chunks 0-3) with pairs of group g-1 and v/out
            # projections threaded between them.
            def emit(u):
                kind = u[0]
                if kind == "pj":
                    qkv_proj(u[1], u[2])
                elif kind == "vp":
                    vproj(u[1], u[2], u[3])
                elif kind == "prq":
                    pair_qk(u[1], u[2])
                elif kind == "pra":
                    pair_av(u[1], u[2])
                elif kind == "op":
                    outproj(u[1], u[2], 0, N2)
                elif kind == "w":
                    warmup(u[1])

            stream = []
            # phase 0: group 0 over all chunks; warmup filler rides between
            # the slots because the x chunks land only every ~2us at the
            # head (the PE would otherwise idle and HAM-rethrottle).
            for c in range(4):
                stream += [("pj", 0, c), ("pj", 6, c), ("w", 3)]
            vp_rest = [("vp", b, jt, 0) for b in range(8) for jt in (0, 1)] + [
                ("vp", b, jt, 1) for b in range(8) for jt in (0, 1)
            ]
            # per-phase pair lists: pairs lag their group by about one
            # phase; in phase 5 the first group-5 pairs start as soon as
            # their chunk slot has run so the drain keeps op filler.
            phase_prs = {
                1: [[], [("pr", 0, 0)], [("pr", 1, 0)], [("pr", 2, 0)]],
                2: [[("pr", 3, 0)], [("pr", 4, 0)],
                    [("pr", 5, 0), ("pr", 0, 1)], [("pr", 6, 0), ("pr", 1, 1)]],
                3: [[("pr", 7, 0), ("pr", 2, 1)], [("pr", 3, 1), ("pr", 4, 1)],
                    [("pr", 5, 1), ("pr", 0, 2)], [("pr", 6, 1), ("pr", 1, 2)]],
                4: [[("pr", 7, 1), ("pr", 2, 2)], [("pr", 3, 2), ("pr", 4, 2)],
                    [("pr", 5, 2), ("pr", 0, 3)], [("pr", 6, 2), ("pr", 1, 3)]],
                5: [[("pr", 7, 2), ("pr", 2, 3), ("pr", 0, 4)],
                    [("pr", 3, 3), ("pr", 4, 3), ("pr", 1, 4), ("pr", 0, 5)],
                    [("pr", 5, 3), ("pr", 2, 4), ("pr", 1, 5)],
                    [("pr", 6, 3), ("pr", 3, 4), ("pr", 2, 5)]],
            }
            vi = 0
            for g in range(1, 6):
                for c in range(4):
                    stream += [("pj", g, c), ("pj", g + 6, c)]
                    stream += phase_prs[g][c]
                    nvp = 2 if g <= 3 else 1
                    for _ in range(nvp):
                        if vi < len(vp_rest):
                            stream.append(vp_rest[vi]); vi += 1
            while vi < len(vp_rest):
                stream.append(vp_rest[vi]); vi += 1
            # drain: remaining pairs with out-projection filler between
            stream += [
                ("pr", 7, 3), ("op", 0, 0), ("pr", 4, 4), ("op", 0, 1),
                ("pr", 3, 5), ("op", 0, 2), ("pr", 5, 4), ("op", 1, 0),
                ("pr", 4, 5), ("op", 0, 3), ("pr", 6, 4), ("op", 1, 1),
                ("pr", 5, 5), ("op", 1, 2), ("pr", 7, 4), ("op", 0, 4),
                ("pr", 6, 5), ("op", 1, 3), ("pr", 7, 5), ("op", 0, 5),
                ("op", 1, 4), ("op", 1, 5),
            ]
            stream += [("op", b2, m) for b2 in (2, 3) for m in range(NPAIR)]

            # split pairs into QK and AV halves, with the AV lagging its
            # QK by ~two units so the EXP latency (scalar queue + ~1us)
            # is fully hidden behind other PE work
            final = []
            pend = []
            for u in stream:
                if u[0] == "pr":
                    final.append(("prq", u[1], u[2]))
                    if pend:
                        final.append(pend.pop(0))
                    pend.append(("pra", u[1], u[2]))
                else:
                    final.append(u)
                    if pend:
                        final.append(pend.pop(0))
            final.extend(pend)

            # bootstrap warmups keep the PE HAM-warm during the initial
            # input DMA wait
            warmup(8)
            for u in final:
                emit(u)

    nc.compile()
    return nc


def _pcontig(mT, r0, nk, c0, c1):
    """Rows r0:r0+nk*128, cols c0:c1 of mT -> [128, nk*(c1-c0)], partition
    p holding its k-tiles contiguously (exact SBUF tile layout)."""
    blk = mT[r0 : r0 + nk * P, c0:c1]
    return np.ascontiguousarray(
        blk.reshape(nk, P, c1 - c0).transpose(1, 0, 2).reshape(P, -1)
    )


def host_in_maps(x, w_qkv, w_out, b_out):
    """Full fp32 inputs -> list of 8 per-core input dicts (bf16)."""
    bf16 = ml_dtypes.bfloat16
    wq = w_qkv[0:D] * SCALE
    wk = w_qkv[D : 2 * D]
    wv = w_qkv[2 * D : 3 * D]
    # interleave q/k blocks per head-pair: group p = [q_p(128), k_p(128)]
    blocks = []
    for p_ in range(NPAIR):
        blocks.append(wq[p_ * P : (p_ + 1) * P])
        blocks.append(wk[p_ * P : (p_ + 1) * P])
    wqkT = np.concatenate(blocks, axis=0).T.astype(bf16)
    wvT = wv.T.astype(bf16)
    woT = w_out.T.astype(bf16)
    bias = np.ascontiguousarray(b_out.reshape(KT, P).T).astype(np.float32)
    shared = {
        "wqk0_0": _pcontig(wqkT, 0, 3, 0, 256),
        "wqk0_1": _pcontig(wqkT, 3 * P, 3, 0, 256),
        "wva": _pcontig(wvT, 0, KT, 0, 512),
        "wvb": _pcontig(wvT, 0, KT, 512, 768),
        "wo0": _pcontig(woT, 0, 3, 0, D),
        "wo1": _pcontig(woT, 3 * P, 3, 0, D),
        "bias": bias,
    }
    for g in range(1, 6):
        shared[f"wqk{g}"] = _pcontig(wqkT, 0, KT, g * 256, (g + 1) * 256)
    in_maps = []
    for c in range(NCORES):
        xc = x[c * BPC : (c + 1) * BPC].reshape(T, D)
        xT = xc.T.astype(bf16)
        m = dict(shared)
        for i in range(3):
            m[f"xc0_{i}"] = _pcontig(xT, i * 2 * P, 2, 0, 394)
        for cc in (1, 2, 3):
            m[f"xc{cc}"] = _pcontig(xT, 0, KT, cc * 394, (cc + 1) * 394)
        in_maps.append(m)
    return in_maps


def host_gather(results):
    """8 per-core {outT: [768, 1576] bf16} -> full [64, 197, 768] fp32."""
    out = np.empty((B, N, D), dtype=np.float32)
    for c in range(NCORES):
        oc = results[c]["outT"].astype(np.float32)  # [D, T]
        out[c * BPC : (c + 1) * BPC] = oc.T.reshape(BPC, N, D)
    return out


_NC_CACHE = []


def kernel(x, w_qkv, w_out, b_out):
    """Full-input entry point: shards batch over 8 NeuronCores, runs the
    Bass kernel, gathers the full [64, 197, 768] fp32 output."""
    if not _NC_CACHE:
        _NC_CACHE.append(build_nc())
    nc = _NC_CACHE[0]
    in_maps = host_in_maps(
        np.asarray(x, dtype=np.float32),
        np.asarray(w_qkv, dtype=np.float32),
        np.asarray(w_out, dtype=np.float32),
        np.asarray(b_out, dtype=np.float32),
    )
    res = run_bass_kernel_spmd(nc, in_maps, core_ids=list(range(NCORES)))
    return host_gather(res.results)


# revision 40
# speedup vs baseline: 1.0176x; 1.0176x over previous
"""Builder + host glue for the ViT attention kernel on 8 trn2 cores.

Reference computation (per batch b):
    qkv = x @ w_qkv.T ; q,k,v split; per head: softmax(q k^T / sqrt(dh)) v
    out = attn @ w_out.T + b_out

Sharding: data-parallel over batch (8 batches per core).

Schedule: group-major. QKV projection runs one 256-col weight group at a
time across all four token chunks, so the head of the kernel only needs
x plus a single weight group in SBUF before the PE saturates (the DMA
feed is pure-bandwidth-bound at ~330 GB/s; front-loading all twelve
groups would stall the PE for ~7us). Attention pairs for group g are
threaded through phase g+1, v projections ride along as soon as wv
lands, and the out-projections fill the tail while the last pairs'
softmax chains drain on Scalar/Vector/GpSimd.

Inputs are pre-arranged on the host into the exact per-partition SBUF
layout of each destination tile, so every DMA is 128 fully contiguous
descriptors.
"""

import numpy as np
import ml_dtypes

import concourse.bass as bass
import concourse.tile as tile
from concourse import bacc, mybir
from concourse.bass_utils import run_bass_kernel_spmd

P = 128
B, N, D = 64, 197, 768
H, DH = 12, 64
NCORES = 8
BPC = B // NCORES          # 8 batches per core
T = BPC * N                # 1576 tokens per core
KT = D // P                # 6 contraction tiles
NPAIR = H // 2             # 6 head pairs
SCALE = DH ** -0.5
VW = (DH + 1) * H          # 780: v columns, ones col LAST per head
N2 = 2 * N                 # 394
JT1 = N - P                # 69: second j-tile size

BF = mybir.dt.bfloat16
F32 = mybir.dt.float32
EXP = mybir.ActivationFunctionType.Exp
IDENT = mybir.ActivationFunctionType.Identity


def build_nc():
    nc = bacc.Bacc(
        "TRN2", target_bir_lowering=False, debug=False, num_devices=NCORES
    )
    d_xc0 = [
        nc.dram_tensor(f"xc0_{i}", [P, 2 * 394], BF, kind="ExternalInput").ap()
        for i in range(3)
    ]
    d_xc = [
        nc.dram_tensor(f"xc{c}", [P, KT * 394], BF, kind="ExternalInput").ap()
        for c in (1, 2, 3)
    ]
    d_wqk0 = [
        nc.dram_tensor(f"wqk0_{i}", [P, 3 * 256], BF, kind="ExternalInput").ap()
        for i in range(2)
    ]
    d_wqk = [
        nc.dram_tensor(f"wqk{g}", [P, KT * 256], BF, kind="ExternalInput").ap()
        for g in range(1, 6)
    ]
    d_wva = nc.dram_tensor("wva", [P, KT * 512], BF, kind="ExternalInput").ap()
    d_wvb = nc.dram_tensor("wvb", [P, KT * 256], BF, kind="ExternalInput").ap()
    d_wo = [
        nc.dram_tensor(f"wo{i}", [P, 3 * D], BF, kind="ExternalInput").ap()
        for i in range(2)
    ]
    bias = nc.dram_tensor("bias", [P, KT], F32, kind="ExternalInput").ap()
    outT = nc.dram_tensor("outT", [D, T], BF, kind="ExternalOutput").ap()

    with tile.TileContext(nc) as tc:
        with (
            tc.tile_pool(name="big", bufs=1) as big,
            tc.tile_pool(name="exp", bufs=8) as sb_exp,
            tc.tile_pool(name="rec", bufs=6) as sb_rec,
            tc.tile_pool(name="bsb", bufs=6) as sb_bsb,
            tc.tile_pool(name="osb", bufs=6) as sb_osb,
            tc.tile_pool(name="ps_pj", bufs=3, space="PSUM") as ps_pj,
            tc.tile_pool(name="ps_att", bufs=5, space="PSUM") as ps_att,
        ):
            # ---- persistent SBUF tiles ----------------------------------
            bias_sb = big.tile([P, KT], F32, tag="bias")
            # x: chunk tiles; c0 split in 3 (2 k-tiles each) for fast start
            x_c0 = [big.tile([P, 2, 394], BF, tag=f"xc0_{i}", name=f"xc0_{i}") for i in range(3)]
            x_c = [big.tile([P, KT, 394], BF, tag=f"xc{c}", name=f"xc{c}") for c in (1, 2, 3)]
            # wqk: per 256-col group; g0 split in 2 (3 k-tiles each)
            wqk_g0 = [big.tile([P, 3, 256], BF, tag=f"wqk0_{i}", name=f"wqk0_{i}") for i in range(2)]
            wqk_g = {
                g: big.tile([P, KT, 256], BF, tag=f"wqk{g}", name=f"wqk{g}")
                for g in range(1, 6)
            }
            # wv split by output col-group: piece 0 = cols 0:512, piece 1 = 512:768
            wv_sb = [
                big.tile([P, KT, 512], BF, tag="wva", name="wva"),
                big.tile([P, KT, 256], BF, tag="wvb", name="wvb"),
            ]
            wo_sb = [big.tile([P, 3, D], BF, tag=f"wo{i}", name=f"wo{i}") for i in range(2)]
            # warmup dummies
            wdum = big.tile([P, P], BF, tag="wdum")
            xdum = big.tile([P, 512], BF, tag="xdum")

            qk_sb = [big.tile([P, T], BF, tag=f"qk{m}", name=f"qk{m}") for m in range(2 * NPAIR)]
            v_sb = [big.tile([P, VW], BF, tag=f"v{i}", name=f"v{i}") for i in range(2 * BPC)]
            at_sb = [
                [big.tile([P, N2], BF, tag=f"at{p}_{b2}", name=f"at{p}_{b2}") for b2 in range(BPC // 2)]
                for p in range(NPAIR)
            ]

            # ---- input DMAs (merged, orchestrated) ----------------------
            def pk(src, kp, cols):
                # dram [p, k*cols] (partition-contiguous) -> [p, k, c]
                return src.rearrange("p (k c) -> p k c", k=kp, c=cols)

            # DMA ring roles (HWDGE DMAs BLOCK their issuing engine until
            # completion): scalar carries the three small x_c0 pieces in
            # parallel with sync's FIFO, which holds everything else in
            # exact consumption order, then the out tiles.
            for i in range(3):
                nc.scalar.dma_start(x_c0[i][:], pk(d_xc0[i], 2, 394))
            for i in range(2):
                nc.sync.dma_start(wqk_g0[i][:], pk(d_wqk0[i], 3, 256))
            nc.sync.dma_start(x_c[0][:], pk(d_xc[0], KT, 394))
            nc.sync.dma_start(wqk_g[1][:], pk(d_wqk[0], KT, 256))
            nc.sync.dma_start(x_c[1][:], pk(d_xc[1], KT, 394))
            nc.sync.dma_start(x_c[2][:], pk(d_xc[2], KT, 394))
            nc.sync.dma_start(wv_sb[0][:], pk(d_wva, KT, 512))
            nc.sync.dma_start(wv_sb[1][:], pk(d_wvb, KT, 256))
            for g in range(2, 6):
                nc.sync.dma_start(wqk_g[g][:], pk(d_wqk[g - 1], KT, 256))
            for i in range(2):
                nc.sync.dma_start(wo_sb[i][:], pk(d_wo[i], 3, D))
            nc.sync.dma_start(bias_sb[:], bias)

            # ---- init memsets -------------------------------------------
            nc.vector.memset(wdum[:], 0.0)
            nc.vector.memset(xdum[:], 0.0)
            for i in range(2 * BPC):
                ones_cols = v_sb[i][:].rearrange("p (h c) -> p h c", c=DH + 1)[
                    :, :, DH : DH + 1
                ]
                nc.gpsimd.memset(ones_cols, 1.0)

            # ---- AP helpers ---------------------------------------------
            def x_ap(k, t0, tl):
                c = t0 // 394
                o = t0 - c * 394
                if c == 0:
                    return x_c0[k // 2][:, k % 2, o : o + tl]
                return x_c[c - 1][:, k, o : o + tl]

            def wqk_ap(k, m):
                # wqkT columns are interleaved per pair: group p holds
                # [q_p (128 cols), k_p (128 cols)] so each pair-slot of the
                # schedule consumes exactly one 256-col weight group.
                g = m % NPAIR
                o = P if m >= NPAIR else 0
                if g == 0:
                    return wqk_g0[k // 3][:, k % 3, o : o + P]
                return wqk_g[g][:, k, o : o + P]

            def wv_ap(k, c0, cl):
                if c0 == 0:
                    return wv_sb[0][:, k, 0:cl]
                return wv_sb[1][:, k, 0:cl]

            def wo_ap(k, m):
                return wo_sb[k // 3][:, k % 3, m * P : (m + 1) * P]

            # ---- engine alternation counters ----------------------------
            alt = {"cast": 0, "act": 0}

            # ---- unit emitters ------------------------------------------
            warm_n = [0]

            def warmup(n):
                psum = ps_pj.tile([P, 512], F32, tag="pj", name="warm")[:, :394]
                for r in range(n):
                    nc.tensor.matmul(
                        psum, wdum[:], xdum[:, 0:394], start=True, stop=True
                    )
                warm_n[0] += 1
                wout = big.tile([P, 8], F32, tag=f"warmout{warm_n[0]}", name="wout")
                nc.vector.tensor_copy(out=wout[:], in_=psum[:, 0:8])

            def qkv_proj(m, c, cast_eng=None):
                t0 = c * 394
                psum = ps_pj.tile([P, 512], F32, tag="pj", name="pj")[:, :394]
                for k in range(KT):
                    nc.tensor.matmul(
                        psum,
                        wqk_ap(k, m),
                        x_ap(k, t0, 394),
                        start=(k == 0),
                        stop=(k == KT - 1),
                    )
                out = qk_sb[m][:, t0 : t0 + 394]
                if cast_eng is None:
                    alt["cast"] += 1
                    cast_eng = "v" if alt["cast"] % 2 == 0 else "s"
                if cast_eng == "v":
                    nc.vector.tensor_copy(out=out, in_=psum)
                else:
                    nc.scalar.copy(out=out, in_=psum)

            def vproj(b, jt, cg, copy_eng=None):
                c0, cl = (0, 512) if cg == 0 else (512, 256)
                r0 = b * N + jt * P
                rl = P if jt == 0 else JT1
                i = 2 * b + jt
                psum = ps_pj.tile([P, 512], F32, tag="pj", name="pjv")[:rl, :cl]
                for k in range(KT):
                    nc.tensor.matmul(
                        psum,
                        x_ap(k, r0, rl),
                        wv_ap(k, c0, cl),
                        start=(k == 0),
                        stop=(k == KT - 1),
                    )
                hs = c0 // DH
                nh = cl // DH
                out_ap = v_sb[i][:rl].rearrange("p (h c) -> p h c", c=DH + 1)[
                    :, hs : hs + nh, 0:DH
                ]
                if copy_eng is None:
                    alt["cast"] += 1
                    copy_eng = "v" if alt["cast"] % 2 == 0 else "s"
                if copy_eng == "s":
                    nc.scalar.copy(
                        out=out_ap, in_=psum.rearrange("p (h c) -> p h c", c=DH)
                    )
                else:
                    nc.vector.tensor_copy(
                        out=out_ap, in_=psum.rearrange("p (h c) -> p h c", c=DH)
                    )

            def outproj(b2, m, t0, tl):
                psum = ps_pj.tile([P, 512], F32, tag="pj", name="op")[:, :tl]
                for p in range(NPAIR):
                    nc.tensor.matmul(
                        psum,
                        wo_ap(p, m),
                        at_sb[p][b2][:, t0 : t0 + tl],
                        start=(p == 0),
                        stop=(p == NPAIR - 1),
                    )
                osb = sb_osb.tile([P, 394], BF, tag="osb", name="osb")[:, :tl]
                alt["act"] += 1
                if alt["act"] % 2 == 0:
                    nc.scalar.activation(osb, psum, IDENT, bias=bias_sb[:, m : m + 1])
                else:
                    nc.vector.tensor_scalar_add(osb, psum, bias_sb[:, m : m + 1])
                nc.sync.dma_start(
                    outT[m * P : (m + 1) * P, b2 * N2 + t0 : b2 * N2 + t0 + tl], osb
                )

            pair_state = {}

            def pair_qk(b, p):
                tb = b * N
                qT = qk_sb[p]
                kTt = qk_sb[NPAIR + p]
                expT = []
                for h in (0, 1):
                    e0 = DH * h
                    ps_s = ps_att.tile([P, N2], F32, tag="att", name="sc")
                    nc.tensor.matmul(
                        ps_s[0:P, 0:N],
                        kTt[e0 : e0 + DH, tb : tb + P],
                        qT[e0 : e0 + DH, tb : tb + N],
                        start=True,
                        stop=True,
                        tile_position=(e0, 0),
                    )
                    nc.tensor.matmul(
                        ps_s[0:JT1, N:N2],
                        kTt[e0 : e0 + DH, tb + P : tb + N],
                        qT[e0 : e0 + DH, tb : tb + N],
                        start=True,
                        stop=True,
                        tile_position=(e0, 0),
                    )
                    e = sb_exp.tile([P, N2], BF, tag="expT", name="expT")
                    nc.scalar.activation(e[:], ps_s[:], EXP)
                    expT.append(e)
                pair_state[(b, p)] = expT

            def pair_av(b, p):
                expT = pair_state.pop((b, p))
                pso = ps_att.tile([P, N2], F32, tag="att", name="o")[0 : DH + 1, :]
                for h in (0, 1):
                    vc = (DH + 1) * (2 * p + h)
                    nc.tensor.matmul(
                        pso[:, N * h : N * h + N],
                        v_sb[2 * b][0:P, vc : vc + DH + 1],
                        expT[h][0:P, 0:N],
                        start=True,
                        stop=False,
                    )
                    nc.tensor.matmul(
                        pso[:, N * h : N * h + N],
                        v_sb[2 * b + 1][0:JT1, vc : vc + DH + 1],
                        expT[h][0:JT1, N:N2],
                        start=False,
                        stop=True,
                    )
                # ones col is LAST per head -> s row = psum partition 64.
                # custom DVE needs base-0 SBUF input: stage s via a copy
                # (alternating engines), then approx-reciprocal.
                s_sb = sb_rec.tile([1, N2], F32, tag="s_sb", name="s_sb")
                alt["cast"] += 1
                if alt["cast"] % 2 == 0:
                    nc.vector.tensor_copy(out=s_sb[:], in_=pso[DH : DH + 1, :])
                else:
                    nc.scalar.copy(out=s_sb[:], in_=pso[DH : DH + 1, :])
                rec = sb_rec.tile([1, N2], F32, tag="rec", name="rec")
                nc.vector.reciprocal_approx_fast(out=rec[:], in_=s_sb[:])
                bsb = sb_bsb.tile([DH, N2], F32, tag="bsb", name="bsb")
                nc.gpsimd.partition_broadcast(bsb[:], rec[:])
                for h in (0, 1):
                    nc.vector.tensor_mul(
                        out=at_sb[p][b // 2][
                            DH * h : DH * h + DH, N * (b % 2) : N * (b % 2) + N
                        ],
                        in0=pso[0:DH, N * h : N * h + N],
                        in1=bsb[:, N * h : N * h + N],
                    )

            # ---- static schedule ----------------------------------------
            # Group-major: phase g emits the 8 qkv units of group g (q and
            # k slots, chunks 0-3) with pairs of group g-1 and v/out
            # projections threaded between them.
            def emit(u):
                kind = u[0]
                if kind == "pj":
                    qkv_proj(u[1], u[2])
                elif kind == "vp":
                    vproj(u[1], u[2], u[3])
                elif kind == "prq":
                    pair_qk(u[1], u[2])
                elif kind == "pra":
                    pair_av(u[1], u[2])
                elif kind == "op":
                    outproj(u[1], u[2], 0, N2)
                elif kind == "w":
                    warmup(u[1])

            stream = []
            # phase 0: group 0 over all chunks; warmup filler rides between
            # the slots because the x chunks land only every ~2us at the
            # head (the PE would otherwise idle and HAM-rethrottle).
            for c in range(4):
                stream += [("pj", 0, c), ("pj", 6, c), ("w", 3)]
            vp_rest = [("vp", b, jt, 0) for b in range(8) for jt in (0, 1)] + [
                ("vp", b, jt, 1) for b in range(8) for jt in (0, 1)
            ]
            # per-phase pair lists: pairs lag their group by about one
            # phase; in phase 5 the first group-5 pairs start as soon as
            # their chunk slot has run so the drain keeps op filler.
            phase_prs = {
                1: [[], [("pr", 0, 0)], [("pr", 1, 0)], [("pr", 2, 0)]],
                2: [[("pr", 3, 0)], [("pr", 4, 0)],
                    [("pr", 5, 0), ("pr", 0, 1)], [("pr", 6, 0), ("pr", 1, 1)]],
                3: [[("pr", 7, 0), ("pr", 2, 1)], [("pr", 3, 1), ("pr", 4, 1)],
                    [("pr", 5, 1), ("pr", 0, 2)], [("pr", 6, 1), ("pr", 1, 2)]],
                4: [[("pr", 7, 1), ("pr", 2, 2)], [("pr", 3, 2), ("pr", 4, 2)],
                    [("pr", 5, 2), ("pr", 0, 3)], [("pr", 6, 2), ("pr", 1, 3)]],
                5: [[("pr", 7, 2), ("pr", 2, 3), ("pr", 0, 4)],
                    [("pr", 3, 3), ("pr", 4, 3), ("pr", 1, 4), ("pr", 0, 5)],
                    [("pr", 5, 3), ("pr", 2, 4), ("pr", 1, 5)],
                    [("pr", 6, 3), ("pr", 3, 4), ("pr", 2, 5)]],
            }
            vi = 0
            for g in range(1, 6):
                for c in range(4):
                    stream += [("pj", g, c), ("pj", g + 6, c)]
                    stream += phase_prs[g][c]
                    nvp = 2 if g <= 3 else 1
                    for _ in range(nvp):
                        if vi < len(vp_rest):
                            stream.append(vp_rest[vi]); vi += 1
            while vi < len(vp_rest):
                stream.append(vp_rest[vi]); vi += 1
            # drain: remaining pairs with out-projection filler between
            stream += [
                ("pr", 7, 3), ("op", 0, 0), ("pr", 4, 4), ("op", 0, 1),
                ("pr", 3, 5), ("op", 0, 2), ("pr", 5, 4), ("op", 1, 0),
                ("pr", 4, 5), ("op", 0, 3), ("pr", 6, 4), ("op", 1, 1),
                ("pr", 5, 5), ("op", 1, 2), ("pr", 7, 4), ("op", 0, 4),
                ("pr", 6, 5), ("op", 1, 3), ("pr", 7, 5), ("op", 0, 5),
                ("op", 1, 4), ("op", 1, 5),
            ]
            stream += [("op", b2, m) for b2 in (2, 3) for m in range(NPAIR)]

            # split pairs into QK and AV halves, with the AV lagging its
            # QK by ~two units so the EXP latency (scalar queue + ~1us)
            # is fully hidden behind other PE work
            final = []
            pend = []
            for u in stream:
                if u[0] == "pr":
                    final.append(("prq", u[1], u[2]))
                    pend.append(("pra", u[1], u[2]))
                    if len(pend) >= 4:
                        final.append(pend.pop(0))
                else:
                    final.append(u)
                    if len(pend) >= 3:
                        final.append(pend.pop(0))
            final.extend(pend)

            # bootstrap warmups keep the PE HAM-warm during the initial
            # input DMA wait
            warmup(8)
            for u in final:
                emit(u)

    nc.compile()
    return nc


def _pcontig(mT, r0, nk, c0, c1):
    """Rows r0:r0+nk*128, cols c0:c1 of mT -> [128, nk*(c1-c0)], partition
    p holding its k-tiles contiguously (exact SBUF tile layout)."""
    blk = mT[r0 : r0 + nk * P, c0:c1]
    return np.ascontiguousarray(
        blk.reshape(nk, P, c1 - c0).transpose(1, 0, 2).reshape(P, -1)
    )


def host_in_maps(x, w_qkv, w_out, b_out):
    """Full fp32 inputs -> list of 8 per-core input dicts (bf16)."""
    bf16 = ml_dtypes.bfloat16
    wq = w_qkv[0:D] * SCALE
    wk = w_qkv[D : 2 * D]
    wv = w_qkv[2 * D : 3 * D]
    # interleave q/k blocks per head-pair: group p = [q_p(128), k_p(128)]
    blocks = []
    for p_ in range(NPAIR):
        blocks.append(wq[p_ * P : (p_ + 1) * P])
        blocks.append(wk[p_ * P : (p_ + 1) * P])
    wqkT = np.concatenate(blocks, axis=0).T.astype(bf16)
    wvT = wv.T.astype(bf16)
    woT = w_out.T.astype(bf16)
    bias = np.ascontiguousarray(b_out.reshape(KT, P).T).astype(np.float32)
    shared = {
        "wqk0_0": _pcontig(wqkT, 0, 3, 0, 256),
        "wqk0_1": _pcontig(wqkT, 3 * P, 3, 0, 256),
        "wva": _pcontig(wvT, 0, KT, 0, 512),
        "wvb": _pcontig(wvT, 0, KT, 512, 768),
        "wo0": _pcontig(woT, 0, 3, 0, D),
        "wo1": _pcontig(woT, 3 * P, 3, 0, D),
        "bias": bias,
    }
    for g in range(1, 6):
        shared[f"wqk{g}"] = _pcontig(wqkT, 0, KT, g * 256, (g + 1) * 256)
    in_maps = []
    for c in range(NCORES):
        xc = x[c * BPC : (c + 1) * BPC].reshape(T, D)
        xT = xc.T.astype(bf16)
        m = dict(shared)
        for i in range(3):
            m[f"xc0_{i}"] = _pcontig(xT, i * 2 * P, 2, 0, 394)
        for cc in (1, 2, 3):
            m[f"xc{cc}"] = _pcontig(xT, 0, KT, cc * 394, (cc + 1) * 394)
        in_maps.append(m)
    return in_maps


def host_gather(results):
    """8 per-core {outT: [768, 1576] bf16} -> full [64, 197, 768] fp32."""
    out = np.empty((B, N, D), dtype=np.float32)
    for c in range(NCORES):
        oc = results[c]["outT"].astype(np.float32)  # [D, T]
        out[c * BPC : (c + 1) * BPC] = oc.T.reshape(BPC, N, D)
    return out


_NC_CACHE = []


def kernel(x, w_qkv, w_out, b_out):
    """Full-input entry point: shards batch over 8 NeuronCores, runs the
    Bass kernel, gathers the full [64, 197, 768] fp32 output."""
    if not _NC_CACHE:
        _NC_CACHE.append(build_nc())
    nc = _NC_CACHE[0]
    in_maps = host_in_maps(
        np.asarray(x, dtype=np.float32),
        np.asarray(w_qkv, dtype=np.float32),
        np.asarray(w_out, dtype=np.float32),
        np.asarray(b_out, dtype=np.float32),
    )
    res = run_bass_kernel_spmd(nc, in_maps, core_ids=list(range(NCORES)))
    return host_gather(res.results)
